# revision 6
# baseline (speedup 1.0000x reference)
"""Trainium2 Bass kernel for nn_CognitiveWorkspaceTransformer.

Math (reference semantics):
    X   = S + concat(w_spoke, w_hub_priv, w_hub_shared, tag)   # full 1088 cover
    out = X @ W_read.T          # (B,T,1024)
    k   = latent @ Wk.T         # cache is fully overwritten by latent
    v   = latent @ Wv.T

Sharding: data-parallel over batch B=8, one batch element per NeuronCore.
All tensors are laid out feature-major on the host (pure layout prep plus a
bf16 downcast, no arithmetic) so the contraction dim lands on SBUF
partitions directly and the PE needs no on-chip transposes.

bf16 everywhere (tolerance is 2e-2; bf16 lands ~5e-3): ~47MB/core HBM
traffic (~131us roofline) vs ~360k PE cycles (~150us @ 2.4GHz) -> the PE
array is the bottleneck; everything else is scheduled to keep it fed:
  - warm-up junk matmuls at t=0 so the HAM clock gate reaches 2.4GHz
    before real work lands (cold matmuls run at 1.2GHz)
  - phase A runs k/v matmuls for the first two slabs (they only need the
    small Wk/Wv/latent loads), covering the W_read/S/wc load ramp
  - consolidated weight loads (W_read 2 DMAs, Wk+Wv packed 1 DMA) since
    HWDGE dma issues block the issuing engine while the ring is full
  - slab i+1 loads are issued BEFORE slab i stores: a store waiting on
    its tile would otherwise block later load issues (in-order queues)
  - tapered slabs [256,768,1024*3] so the first adds/out-matmuls start
    early; big slabs use 2KB-contiguous DMA runs
  - j-outer/h-inner so each 128x128 stationary X^T chunk is loaded once
  - 2-bank PSUM tiles [128,1024]; ONE wide PSUM->SBUF cast-copy per
    out/k/v tile (out,v on DVE; k on ACT); paired [256,1024] stores
"""

import numpy as np
import ml_dtypes

import concourse.bacc as bacc
import concourse.mybir as mybir
import concourse.tile as tile
from concourse.bass_utils import run_bass_kernel_spmd

B, T, D_STATE, D_MODEL, D_LATENT = 8, 4096, 1088, 1024, 128
N_CORES = 8
P = 128
F32 = mybir.dt.float32
BF16 = mybir.dt.bfloat16

# feature chunks of the contraction dim (1088 = 8*128 + 64)
R_CHUNKS = [(j * 128, min(128, D_STATE - j * 128)) for j in range((D_STATE + 127) // 128)]
NJ = len(R_CHUNKS)

_NC_CACHE = {}

SLABS = [256, 768, 1024, 1024, 1024]
KV_PHASE_A = 2  # number of leading slabs whose k/v runs in phase A


def build_nc(mm_dt=BF16, out_dt=BF16, in_bufs=3, wc_bufs=2, out_bufs=2,
             kv0_bufs=4, warmup_mms=12):
    """Build + compile the per-core Bass program (identical on all cores)."""
    assert sum(SLABS) == T
    max_sz = max(SLABS)

    nc = bacc.Bacc("TRN2", target_bir_lowering=False, debug=False, num_devices=N_CORES)

    # feature-major inputs: sT/wcT [1088, T], latT [128, T], wkvt [128,2,1024]
    st_d = nc.dram_tensor("st", [D_STATE, T], mm_dt, kind="ExternalInput").ap()
    wct_d = nc.dram_tensor("wct", [D_STATE, T], mm_dt, kind="ExternalInput").ap()
    latt_d = nc.dram_tensor("latt", [D_LATENT, T], mm_dt, kind="ExternalInput").ap()
    wrt_d = nc.dram_tensor("wrt", [D_STATE, D_MODEL], mm_dt, kind="ExternalInput").ap()
    wkvt_d = nc.dram_tensor("wkvt", [D_LATENT, 2, D_MODEL], mm_dt,
                            kind="ExternalInput").ap()
    out_d = nc.dram_tensor("out", [T, D_MODEL], out_dt, kind="ExternalOutput").ap()
    k_d = nc.dram_tensor("k", [T, D_MODEL], out_dt, kind="ExternalOutput").ap()
    v_d = nc.dram_tensor("v", [T, D_MODEL], out_dt, kind="ExternalOutput").ap()

    with tile.TileContext(nc) as tc:
        with (
            tc.tile_pool(name="weights", bufs=1) as wpool,
            tc.tile_pool(name="ins", bufs=in_bufs) as inpool,
            tc.tile_pool(name="wcp", bufs=wc_bufs) as wcpool,
            tc.tile_pool(name="outs", bufs=out_bufs) as outpool,
            tc.tile_pool(name="kv0", bufs=kv0_bufs) as kv0pool,
            tc.tile_pool(name="psum_out", bufs=2, space="PSUM") as pout_pool,
            tc.tile_pool(name="psum_kv", bufs=2, space="PSUM") as pkv_pool,
        ):
            # scalar queue: small weights + whole latT first -> k/v matmuls
            # can start ~2us in, while W_read/S/wc still stream
            wkv_t = wpool.tile([D_LATENT, 2, D_MODEL], mm_dt, tag="wkv")
            nc.scalar.dma_start(wkv_t[:], wkvt_d[:])
            lt = wpool.tile([D_LATENT, T], mm_dt, tag="lt")
            nc.scalar.dma_start(lt[:], latt_d[:])
            ltr = lt[:]

            # sync queue: W_read as one rearranged tile (+ separate tag rows)
            wr_all = wpool.tile([P, NJ, D_MODEL], mm_dt, tag="wr")
            nc.sync.dma_start(
                wr_all[:, 0:8, :],
                wrt_d[0:1024, :].rearrange("(j p) n -> p j n", p=P))
            nc.sync.dma_start(wr_all[0:64, 8, :], wrt_d[1024:1088, :])

            # HAM warm-up: junk matmuls on a zeroed scratch tile keep the PE
            # busy through the clock-gate window while the first loads land
            if warmup_mms:
                scratch = wpool.tile([P, 512], mm_dt, tag="scratch")
                nc.vector.memset(scratch[:], 0.0)
                pj = pout_pool.tile([P, D_MODEL], F32, tag="pout", name="pjunk")
                for w in range(warmup_mms):
                    nc.tensor.matmul(pj[:, 0:512], scratch[:, 0:P],
                                     scratch[:, 0:512], start=True, stop=True)
                # keep the junk matmuls live
                keep = wpool.tile([1, 8], F32, tag="keep")
                nc.vector.tensor_copy(keep[:], pj[0:1, 0:8])

            def emit_kv(ts_abs, k_sb, v_sb):
                pk = pkv_pool.tile([P, D_MODEL], F32, tag="pkv", name="pk")
                for h in range(2):
                    nc.tensor.matmul(
                        pk[:, h * 512 : h * 512 + 512],
                        ltr[:, ts_abs : ts_abs + P],
                        wkv_t[:, 0, h * 512 : h * 512 + 512],
                        start=True, stop=True)
                nc.scalar.copy(k_sb, pk[:])
                pv = pkv_pool.tile([P, D_MODEL], F32, tag="pkv", name="pv")
                for h in range(2):
                    nc.tensor.matmul(
                        pv[:, h * 512 : h * 512 + 512],
                        ltr[:, ts_abs : ts_abs + P],
                        wkv_t[:, 1, h * 512 : h * 512 + 512],
                        start=True, stop=True)
                nc.vector.tensor_copy(v_sb, pv[:])

            def emit_out(xr, ts0, out_sb):
                po = pout_pool.tile([P, D_MODEL], F32, tag="pout", name="po")
                for j, (r0, rw) in enumerate(R_CHUNKS):
                    for h in range(2):
                        nc.tensor.matmul(
                            po[:, h * 512 : h * 512 + 512],
                            xr[0:rw, j, ts0 : ts0 + P],
                            wr_all[0:rw, j, h * 512 : h * 512 + 512],
                            start=(j == 0),
                            stop=(j == NJ - 1),
                        )
                nc.vector.tensor_copy(out_sb, po[:])

            def pair_store(eng, dram, tl, row0):
                dst = dram[row0 : row0 + 2 * P, :].rearrange("(g p) d -> p g d", p=P)
                eng.dma_start(dst, tl[:])

            def issue_loads(it):
                sz = SLABS[it]
                t0 = sum(SLABS[:it])
                xt = inpool.tile([P, NJ, sz], mm_dt, tag="x", name="xt",
                                 padded_shape=[P, NJ, max_sz])
                wc = wcpool.tile([P, NJ, sz], mm_dt, tag="wc", name="wc",
                                 padded_shape=[P, NJ, max_sz])
                nc.sync.dma_start(
                    xt[:, 0:8, :],
                    st_d[0:1024, t0 : t0 + sz].rearrange("(j p) t -> p j t", p=P))
                nc.sync.dma_start(xt[0:64, 8, :], st_d[1024:1088, t0 : t0 + sz])
                nc.scalar.dma_start(
                    wc[:, 0:8, :],
                    wct_d[0:1024, t0 : t0 + sz].rearrange("(j p) t -> p j t", p=P))
                nc.scalar.dma_start(wc[0:64, 8, :], wct_d[1024:1088, t0 : t0 + sz])
                return xt, wc

            # preload slabs 0 and 1
            slab_tiles = {0: issue_loads(0), 1: issue_loads(1)}

            # ---- phase A: k/v for slabs 0..KV_PHASE_A-1 over the ramp ----
            kv_pairs = []  # (row0, k_pair, v_pair) pending stores
            for it in range(KV_PHASE_A):
                t0 = sum(SLABS[:it])
                for g in range(SLABS[it] // P):
                    if g % 2 == 0:
                        kp = kv0pool.tile([P, 2, D_MODEL], out_dt, tag="k0",
                                          name="k0_pr")
                        vp = kv0pool.tile([P, 2, D_MODEL], out_dt, tag="v0",
                                          name="v0_pr")
                    emit_kv(t0 + g * P, kp[:, g % 2, :], vp[:, g % 2, :])
                    if g % 2 == 1:
                        kv_pairs.append((t0 + (g - 1) * P, kp, vp))
            # all phase-A kv stores on scalar (sync keeps pure loads flowing)
            for row0, kp, vp in kv_pairs:
                pair_store(nc.scalar, k_d, kp, row0)
                pair_store(nc.scalar, v_d, vp, row0)

            # ---- main slab loop ----
            for it, sz in enumerate(SLABS):
                t0 = sum(SLABS[:it])
                ng = sz // P
                xt, wc = slab_tiles.pop(it)
                xr = xt[:]

                # prefetch next slab BEFORE this slab's stores hit the queues
                if it + 1 < len(SLABS):
                    slab_tiles[it + 1] = issue_loads(it + 1)

                for g in range(ng):
                    sl = slice(g * P, (g + 1) * P)
                    nc.vector.tensor_add(xr[:, :, sl], xt[:, :, sl], wc[:, :, sl])

                kv_here = it >= KV_PHASE_A
                tiles = None
                for g in range(ng):
                    if g % 2 == 0:
                        tiles = [outpool.tile([P, 2, D_MODEL], out_dt, tag="out",
                                              name="out_pr")]
                        if kv_here:
                            tiles.append(outpool.tile([P, 2, D_MODEL], out_dt,
                                                      tag="k", name="k_pr"))
                            tiles.append(outpool.tile([P, 2, D_MODEL], out_dt,
                                                      tag="v", name="v_pr"))
                    emit_out(xr, g * P, tiles[0][:, g % 2, :])
                    if kv_here:
                        emit_kv(t0 + g * P, tiles[1][:, g % 2, :],
                                tiles[2][:, g % 2, :])
                    if g % 2 == 1:
                        row0 = t0 + (g - 1) * P
                        eng = [nc.scalar, nc.sync] if (g // 2) % 2 == 0 else \
                              [nc.sync, nc.scalar]
                        pair_store(eng[0], out_d, tiles[0], row0)
                        if kv_here:
                            pair_store(eng[1], k_d, tiles[1], row0)
                            pair_store(eng[0], v_d, tiles[2], row0)

    nc.compile()
    return nc


def _get_nc(**kw):
    key = tuple(sorted(kw.items()))
    if key not in _NC_CACHE:
        _NC_CACHE[key] = build_nc(**kw)
    return _NC_CACHE[key]


def make_in_maps(S, w_spoke, w_hub_priv, w_hub_shared, tag, W_read, cache, latent,
                 Wk, Wv):
    # host-side layout prep only (shard over batch, feature-major transposes,
    # bf16 downcast)
    bf = ml_dtypes.bfloat16
    wcat = np.concatenate(
        [np.asarray(w_spoke, np.float32), np.asarray(w_hub_priv, np.float32),
         np.asarray(w_hub_shared, np.float32), np.asarray(tag, np.float32)],
        axis=-1,
    )
    sT = np.ascontiguousarray(np.asarray(S, np.float32).transpose(0, 2, 1)).astype(bf)
    wcT = np.ascontiguousarray(wcat.transpose(0, 2, 1)).astype(bf)
    latT = np.ascontiguousarray(
        np.asarray(latent, np.float32).transpose(0, 2, 1)).astype(bf)
    wrt = np.ascontiguousarray(np.asarray(W_read, np.float32).T).astype(bf)
    wkvt = np.ascontiguousarray(
        np.stack([np.asarray(Wk, np.float32).T, np.asarray(Wv, np.float32).T],
                 axis=1)).astype(bf)
    return [
        {"st": sT[i], "wct": wcT[i], "latt": latT[i], "wrt": wrt, "wkvt": wkvt}
        for i in range(N_CORES)
    ]


def kernel(S, w_spoke, w_hub_priv, w_hub_shared, tag, W_read, cache, latent, Wk, Wv,
           **build_kw):
    in_maps = make_in_maps(S, w_spoke, w_hub_priv, w_hub_shared, tag, W_read, cache,
                           latent, Wk, Wv)
    nc = _get_nc(**build_kw)
    res = run_bass_kernel_spmd(nc, in_maps, list(range(N_CORES)))
    out = np.stack([res.results[i]["out"].astype(np.float32) for i in range(N_CORES)])
    k = np.stack([res.results[i]["k"].astype(np.float32) for i in range(N_CORES)])
    v = np.stack([res.results[i]["v"].astype(np.float32) for i in range(N_CORES)])
    return (out, k, v)


# revision 7
# speedup vs baseline: 1.0583x; 1.0583x over previous
"""Trainium2 Bass kernel for nn_CognitiveWorkspaceTransformer.

Math (reference semantics):
    X   = S + concat(w_spoke, w_hub_priv, w_hub_shared, tag)   # full 1088 cover
    out = X @ W_read.T          # (B,T,1024)
    k   = latent @ Wk.T         # cache is fully overwritten by latent
    v   = latent @ Wv.T

Sharding: data-parallel over batch B=8, one batch element per NeuronCore.
All tensors are laid out feature-major on the host (pure layout prep plus a
bf16 downcast, no arithmetic) so the contraction dim lands on SBUF
partitions directly and the PE needs no on-chip transposes.

bf16 everywhere (tolerance is 2e-2; bf16 lands ~5e-3): ~47MB/core HBM
traffic (~131us roofline) vs ~360k PE cycles (~150us @ 2.4GHz) -> the PE
array is the bottleneck; everything else is scheduled to keep it fed:
  - warm-up junk matmuls at t=0 so the HAM clock gate reaches 2.4GHz
    before real work lands (cold matmuls run at 1.2GHz)
  - phase A runs k/v matmuls for the first two slabs (they only need the
    small Wk/Wv/latent loads), covering the W_read/S/wc load ramp
  - consolidated weight loads (W_read 2 DMAs, Wk+Wv packed 1 DMA) since
    HWDGE dma issues block the issuing engine while the ring is full
  - slab i+1 loads are issued BEFORE slab i stores: a store waiting on
    its tile would otherwise block later load issues (in-order queues)
  - tapered slabs [256,768,1024*3] so the first adds/out-matmuls start
    early; big slabs use 2KB-contiguous DMA runs
  - j-outer/h-inner so each 128x128 stationary X^T chunk is loaded once
  - 2-bank PSUM tiles [128,1024]; ONE wide PSUM->SBUF cast-copy per
    out/k/v tile (out,v on DVE; k on ACT); paired [256,1024] stores
"""

import numpy as np
import ml_dtypes

import concourse.bacc as bacc
import concourse.mybir as mybir
import concourse.tile as tile
from concourse.bass_utils import run_bass_kernel_spmd

B, T, D_STATE, D_MODEL, D_LATENT = 8, 4096, 1088, 1024, 128
N_CORES = 8
P = 128
F32 = mybir.dt.float32
BF16 = mybir.dt.bfloat16

# feature chunks of the contraction dim (1088 = 8*128 + 64)
R_CHUNKS = [(j * 128, min(128, D_STATE - j * 128)) for j in range((D_STATE + 127) // 128)]
NJ = len(R_CHUNKS)

_NC_CACHE = {}

SLABS = [256, 768, 1024, 1024, 1024]
KV_PHASE_A = 2  # number of leading slabs whose k/v runs in phase A


def build_nc(mm_dt=BF16, out_dt=BF16, in_bufs=3, wc_bufs=2, out_bufs=2,
             kv0_bufs=4, warmup_mms=12):
    """Build + compile the per-core Bass program (identical on all cores)."""
    assert sum(SLABS) == T
    max_sz = max(SLABS)

    nc = bacc.Bacc("TRN2", target_bir_lowering=False, debug=False, num_devices=N_CORES)

    # feature-major inputs: sT/wcT [1088, T], latT [128, T], wkvt [128,2,1024]
    st_d = nc.dram_tensor("st", [D_STATE, T], mm_dt, kind="ExternalInput").ap()
    wct_d = nc.dram_tensor("wct", [D_STATE, T], mm_dt, kind="ExternalInput").ap()
    latt_d = nc.dram_tensor("latt", [D_LATENT, T], mm_dt, kind="ExternalInput").ap()
    wrt_d = nc.dram_tensor("wrt", [D_STATE, D_MODEL], mm_dt, kind="ExternalInput").ap()
    wkvt_d = nc.dram_tensor("wkvt", [D_LATENT, 2, D_MODEL], mm_dt,
                            kind="ExternalInput").ap()
    out_d = nc.dram_tensor("out", [T, D_MODEL], out_dt, kind="ExternalOutput").ap()
    k_d = nc.dram_tensor("k", [T, D_MODEL], out_dt, kind="ExternalOutput").ap()
    v_d = nc.dram_tensor("v", [T, D_MODEL], out_dt, kind="ExternalOutput").ap()

    with tile.TileContext(nc) as tc:
        with (
            tc.tile_pool(name="weights", bufs=1) as wpool,
            tc.tile_pool(name="ins", bufs=in_bufs) as inpool,
            tc.tile_pool(name="wcp", bufs=wc_bufs) as wcpool,
            tc.tile_pool(name="outs", bufs=out_bufs) as outpool,
            tc.tile_pool(name="kv0", bufs=kv0_bufs) as kv0pool,
            tc.tile_pool(name="psum_out", bufs=2, space="PSUM") as pout_pool,
            tc.tile_pool(name="psum_kv", bufs=2, space="PSUM") as pkv_pool,
        ):
            # scalar queue: small weights + whole latT first -> k/v matmuls
            # can start ~2us in, while W_read/S/wc still stream
            wkv_t = wpool.tile([D_LATENT, 2, D_MODEL], mm_dt, tag="wkv")
            nc.scalar.dma_start(wkv_t[:], wkvt_d[:])
            lt = wpool.tile([D_LATENT, T], mm_dt, tag="lt")
            nc.scalar.dma_start(lt[:], latt_d[:])
            ltr = lt[:]

            # sync queue: W_read as one rearranged tile (+ separate tag rows)
            wr_all = wpool.tile([P, NJ, D_MODEL], mm_dt, tag="wr")
            nc.sync.dma_start(
                wr_all[:, 0:8, :],
                wrt_d[0:1024, :].rearrange("(j p) n -> p j n", p=P))
            nc.sync.dma_start(wr_all[0:64, 8, :], wrt_d[1024:1088, :])

            # HAM warm-up: junk matmuls on a zeroed scratch tile keep the PE
            # busy through the clock-gate window while the first loads land
            if warmup_mms:
                scratch = wpool.tile([P, 512], mm_dt, tag="scratch")
                nc.vector.memset(scratch[:], 0.0)
                pj = pout_pool.tile([P, D_MODEL], F32, tag="pout", name="pjunk")
                for w in range(warmup_mms):
                    nc.tensor.matmul(pj[:, 0:512], scratch[:, 0:P],
                                     scratch[:, 0:512], start=True, stop=True)
                # keep the junk matmuls live
                keep = wpool.tile([1, 8], F32, tag="keep")
                nc.vector.tensor_copy(keep[:], pj[0:1, 0:8])

            def emit_kv(ts_abs, k_sb, v_sb):
                pk = pkv_pool.tile([P, D_MODEL], F32, tag="pkv", name="pk")
                for h in range(2):
                    nc.tensor.matmul(
                        pk[:, h * 512 : h * 512 + 512],
                        ltr[:, ts_abs : ts_abs + P],
                        wkv_t[:, 0, h * 512 : h * 512 + 512],
                        start=True, stop=True)
                nc.scalar.copy(k_sb, pk[:])
                pv = pkv_pool.tile([P, D_MODEL], F32, tag="pkv", name="pv")
                for h in range(2):
                    nc.tensor.matmul(
                        pv[:, h * 512 : h * 512 + 512],
                        ltr[:, ts_abs : ts_abs + P],
                        wkv_t[:, 1, h * 512 : h * 512 + 512],
                        start=True, stop=True)
                nc.vector.tensor_copy(v_sb, pv[:])

            def emit_out(xr, ts0, out_sb):
                po = pout_pool.tile([P, D_MODEL], F32, tag="pout", name="po")
                for j, (r0, rw) in enumerate(R_CHUNKS):
                    for h in range(2):
                        nc.tensor.matmul(
                            po[:, h * 512 : h * 512 + 512],
                            xr[0:rw, j, ts0 : ts0 + P],
                            wr_all[0:rw, j, h * 512 : h * 512 + 512],
                            start=(j == 0),
                            stop=(j == NJ - 1),
                        )
                nc.vector.tensor_copy(out_sb, po[:])

            def pair_store(eng, dram, tl, row0):
                dst = dram[row0 : row0 + 2 * P, :].rearrange("(g p) d -> p g d", p=P)
                eng.dma_start(dst, tl[:])

            def issue_loads(it):
                sz = SLABS[it]
                t0 = sum(SLABS[:it])
                xt = inpool.tile([P, NJ, sz], mm_dt, tag="x", name="xt",
                                 padded_shape=[P, NJ, max_sz])
                wc = wcpool.tile([P, NJ, sz], mm_dt, tag="wc", name="wc",
                                 padded_shape=[P, NJ, max_sz])
                nc.sync.dma_start(
                    xt[:, 0:8, :],
                    st_d[0:1024, t0 : t0 + sz].rearrange("(j p) t -> p j t", p=P))
                nc.sync.dma_start(xt[0:64, 8, :], st_d[1024:1088, t0 : t0 + sz])
                nc.scalar.dma_start(
                    wc[:, 0:8, :],
                    wct_d[0:1024, t0 : t0 + sz].rearrange("(j p) t -> p j t", p=P))
                nc.scalar.dma_start(wc[0:64, 8, :], wct_d[1024:1088, t0 : t0 + sz])
                return xt, wc

            # preload slabs 0 and 1
            slab_tiles = {0: issue_loads(0), 1: issue_loads(1)}

            # ---- phase A: k/v for slabs 0..KV_PHASE_A-1 over the ramp ----
            kv_pairs = []  # (row0, k_pair, v_pair) pending stores
            for it in range(KV_PHASE_A):
                t0 = sum(SLABS[:it])
                for g in range(SLABS[it] // P):
                    if g % 2 == 0:
                        kp = kv0pool.tile([P, 2, D_MODEL], out_dt, tag="k0",
                                          name="k0_pr")
                        vp = kv0pool.tile([P, 2, D_MODEL], out_dt, tag="v0",
                                          name="v0_pr")
                    emit_kv(t0 + g * P, kp[:, g % 2, :], vp[:, g % 2, :])
                    if g % 2 == 1:
                        kv_pairs.append((t0 + (g - 1) * P, kp, vp))
            # all phase-A kv stores on scalar (sync keeps pure loads flowing)
            for row0, kp, vp in kv_pairs:
                pair_store(nc.scalar, k_d, kp, row0)
                pair_store(nc.scalar, v_d, vp, row0)

            # ---- main slab loop ----
            for it, sz in enumerate(SLABS):
                t0 = sum(SLABS[:it])
                ng = sz // P
                xt, wc = slab_tiles.pop(it)
                xr = xt[:]

                # prefetch next slab BEFORE this slab's stores hit the queues
                if it + 1 < len(SLABS) and it + 1 not in slab_tiles:
                    slab_tiles[it + 1] = issue_loads(it + 1)

                for g in range(ng):
                    sl = slice(g * P, (g + 1) * P)
                    nc.vector.tensor_add(xr[:, :, sl], xt[:, :, sl], wc[:, :, sl])

                kv_here = it >= KV_PHASE_A
                tiles = None
                for g in range(ng):
                    if g % 2 == 0:
                        tiles = [outpool.tile([P, 2, D_MODEL], out_dt, tag="out",
                                              name="out_pr")]
                        if kv_here:
                            tiles.append(outpool.tile([P, 2, D_MODEL], out_dt,
                                                      tag="k", name="k_pr"))
                            tiles.append(outpool.tile([P, 2, D_MODEL], out_dt,
                                                      tag="v", name="v_pr"))
                    emit_out(xr, g * P, tiles[0][:, g % 2, :])
                    if kv_here:
                        emit_kv(t0 + g * P, tiles[1][:, g % 2, :],
                                tiles[2][:, g % 2, :])
                    if g % 2 == 1:
                        row0 = t0 + (g - 1) * P
                        eng = [nc.scalar, nc.sync] if (g // 2) % 2 == 0 else \
                              [nc.sync, nc.scalar]
                        pair_store(eng[0], out_d, tiles[0], row0)
                        if kv_here:
                            pair_store(eng[1], k_d, tiles[1], row0)
                            pair_store(eng[0], v_d, tiles[2], row0)

    nc.compile()
    return nc


def _get_nc(**kw):
    key = tuple(sorted(kw.items()))
    if key not in _NC_CACHE:
        _NC_CACHE[key] = build_nc(**kw)
    return _NC_CACHE[key]


def make_in_maps(S, w_spoke, w_hub_priv, w_hub_shared, tag, W_read, cache, latent,
                 Wk, Wv):
    # host-side layout prep only (shard over batch, feature-major transposes,
    # bf16 downcast)
    bf = ml_dtypes.bfloat16
    wcat = np.concatenate(
        [np.asarray(w_spoke, np.float32), np.asarray(w_hub_priv, np.float32),
         np.asarray(w_hub_shared, np.float32), np.asarray(tag, np.float32)],
        axis=-1,
    )
    sT = np.ascontiguousarray(np.asarray(S, np.float32).transpose(0, 2, 1)).astype(bf)
    wcT = np.ascontiguousarray(wcat.transpose(0, 2, 1)).astype(bf)
    latT = np.ascontiguousarray(
        np.asarray(latent, np.float32).transpose(0, 2, 1)).astype(bf)
    wrt = np.ascontiguousarray(np.asarray(W_read, np.float32).T).astype(bf)
    wkvt = np.ascontiguousarray(
        np.stack([np.asarray(Wk, np.float32).T, np.asarray(Wv, np.float32).T],
                 axis=1)).astype(bf)
    return [
        {"st": sT[i], "wct": wcT[i], "latt": latT[i], "wrt": wrt, "wkvt": wkvt}
        for i in range(N_CORES)
    ]


def kernel(S, w_spoke, w_hub_priv, w_hub_shared, tag, W_read, cache, latent, Wk, Wv,
           **build_kw):
    in_maps = make_in_maps(S, w_spoke, w_hub_priv, w_hub_shared, tag, W_read, cache,
                           latent, Wk, Wv)
    nc = _get_nc(**build_kw)
    res = run_bass_kernel_spmd(nc, in_maps, list(range(N_CORES)))
    out = np.stack([res.results[i]["out"].astype(np.float32) for i in range(N_CORES)])
    k = np.stack([res.results[i]["k"].astype(np.float32) for i in range(N_CORES)])
    v = np.stack([res.results[i]["v"].astype(np.float32) for i in range(N_CORES)])
    return (out, k, v)


# revision 8
# speedup vs baseline: 1.0968x; 1.0364x over previous
"""Trainium2 Bass kernel for nn_CognitiveWorkspaceTransformer.

Math (reference semantics):
    X   = S + concat(w_spoke, w_hub_priv, w_hub_shared, tag)   # full 1088 cover
    out = X @ W_read.T          # (B,T,1024)
    k   = latent @ Wk.T         # cache is fully overwritten by latent
    v   = latent @ Wv.T

Sharding: data-parallel over batch B=8, one batch element per NeuronCore.
All tensors are laid out feature-major on the host (pure layout prep plus a
bf16 downcast, no arithmetic) so the contraction dim lands on SBUF
partitions directly and the PE needs no on-chip transposes.

bf16 everywhere (tolerance is 2e-2; bf16 lands ~5e-3): ~47MB/core HBM
traffic (~131us roofline) vs ~360k PE cycles (~150us @ 2.4GHz) -> the PE
array is the bottleneck; everything else is scheduled to keep it fed:
  - a few warm-up junk matmuls at t=0 so the HAM clock gate reaches
    2.4GHz before real work lands (cold matmuls run at 1.2GHz)
  - slabs 0-1 are small (256/512 tokens) and run k/v matmuls (needing
    only the small Wk/Wv + latent loads) BEFORE their out-matmuls, so
    the PE has work during the W_read/S/wc load ramp; their k/v stores
    are deferred to slabs 2-3 so ramp loads keep the full HBM bandwidth
  - consolidated weight loads (W_read 2 DMAs, Wk+Wv packed 1 DMA) since
    HWDGE dma issues block the issuing engine while the ring is full
  - slab i+1 loads are issued BEFORE slab i stores: a store waiting on
    its tile would otherwise block later load issues (in-order queues)
  - j-outer/h-inner so each 128x128 stationary X^T chunk is loaded once
  - 2-bank PSUM tiles [128,1024]; ONE wide PSUM->SBUF cast-copy per
    out/k/v tile (out,v on DVE; k on ACT); paired [256,1024] stores
"""

import numpy as np
import ml_dtypes

import concourse.bacc as bacc
import concourse.mybir as mybir
import concourse.tile as tile
from concourse.bass_utils import run_bass_kernel_spmd

B, T, D_STATE, D_MODEL, D_LATENT = 8, 4096, 1088, 1024, 128
N_CORES = 8
P = 128
F32 = mybir.dt.float32
BF16 = mybir.dt.bfloat16

# feature chunks of the contraction dim (1088 = 8*128 + 64)
R_CHUNKS = [(j * 128, min(128, D_STATE - j * 128)) for j in range((D_STATE + 127) // 128)]
NJ = len(R_CHUNKS)

_NC_CACHE = {}

SLABS = [256, 512, 1024, 1024, 1280]
KV_FIRST = 2  # leading slabs run k/v before out, with deferred stores


def build_nc(mm_dt=BF16, out_dt=BF16, in_bufs=2, wc_bufs=2, out_bufs=2,
             warmup_mms=6):
    """Build + compile the per-core Bass program (identical on all cores)."""
    assert sum(SLABS) == T
    max_sz = max(SLABS)

    nc = bacc.Bacc("TRN2", target_bir_lowering=False, debug=False, num_devices=N_CORES)

    # feature-major inputs: sT/wcT [1088, T], latT [128, T], wkvt [128,2,1024]
    st_d = nc.dram_tensor("st", [D_STATE, T], mm_dt, kind="ExternalInput").ap()
    wct_d = nc.dram_tensor("wct", [D_STATE, T], mm_dt, kind="ExternalInput").ap()
    latt_d = nc.dram_tensor("latt", [D_LATENT, T], mm_dt, kind="ExternalInput").ap()
    wrt_d = nc.dram_tensor("wrt", [D_STATE, D_MODEL], mm_dt, kind="ExternalInput").ap()
    wkvt_d = nc.dram_tensor("wkvt", [D_LATENT, 2, D_MODEL], mm_dt,
                            kind="ExternalInput").ap()
    out_d = nc.dram_tensor("out", [T, D_MODEL], out_dt, kind="ExternalOutput").ap()
    k_d = nc.dram_tensor("k", [T, D_MODEL], out_dt, kind="ExternalOutput").ap()
    v_d = nc.dram_tensor("v", [T, D_MODEL], out_dt, kind="ExternalOutput").ap()

    with tile.TileContext(nc) as tc:
        with (
            tc.tile_pool(name="weights", bufs=1) as wpool,
            tc.tile_pool(name="ins", bufs=in_bufs) as inpool,
            tc.tile_pool(name="wcp", bufs=wc_bufs) as wcpool,
            tc.tile_pool(name="outs", bufs=out_bufs) as outpool,
            tc.tile_pool(name="kv0", bufs=3) as kv0pool,
            tc.tile_pool(name="psum_out", bufs=2, space="PSUM") as pout_pool,
            tc.tile_pool(name="psum_kv", bufs=2, space="PSUM") as pkv_pool,
        ):
            # scalar queue: small weights + whole latT first -> k/v matmuls
            # can start ~2us in, while W_read/S/wc still stream
            wkv_t = wpool.tile([D_LATENT, 2, D_MODEL], mm_dt, tag="wkv")
            nc.scalar.dma_start(wkv_t[:], wkvt_d[:])
            lt = wpool.tile([D_LATENT, T], mm_dt, tag="lt")
            nc.scalar.dma_start(lt[:], latt_d[:])
            ltr = lt[:]

            # sync queue: W_read as one rearranged tile (+ separate tag rows)
            wr_all = wpool.tile([P, NJ, D_MODEL], mm_dt, tag="wr")
            nc.sync.dma_start(
                wr_all[:, 0:8, :],
                wrt_d[0:1024, :].rearrange("(j p) n -> p j n", p=P))
            nc.sync.dma_start(wr_all[0:64, 8, :], wrt_d[1024:1088, :])

            # HAM warm-up: junk matmuls on a zeroed scratch tile keep the PE
            # busy through the clock-gate window while the first loads land
            if warmup_mms:
                scratch = wpool.tile([P, 512], mm_dt, tag="scratch")
                nc.vector.memset(scratch[:], 0.0)
                pj = pout_pool.tile([P, D_MODEL], F32, tag="pout", name="pjunk")
                for w in range(warmup_mms):
                    nc.tensor.matmul(pj[:, 0:512], scratch[:, 0:P],
                                     scratch[:, 0:512], start=True, stop=True)
                # keep the junk matmuls live
                keep = wpool.tile([1, 8], F32, tag="keep")
                nc.vector.tensor_copy(keep[:], pj[0:1, 0:8])

            def emit_kv(ts_abs, k_sb, v_sb):
                pk = pkv_pool.tile([P, D_MODEL], F32, tag="pkv", name="pk")
                for h in range(2):
                    nc.tensor.matmul(
                        pk[:, h * 512 : h * 512 + 512],
                        ltr[:, ts_abs : ts_abs + P],
                        wkv_t[:, 0, h * 512 : h * 512 + 512],
                        start=True, stop=True)
                nc.scalar.copy(k_sb, pk[:])
                pv = pkv_pool.tile([P, D_MODEL], F32, tag="pkv", name="pv")
                for h in range(2):
                    nc.tensor.matmul(
                        pv[:, h * 512 : h * 512 + 512],
                        ltr[:, ts_abs : ts_abs + P],
                        wkv_t[:, 1, h * 512 : h * 512 + 512],
                        start=True, stop=True)
                nc.vector.tensor_copy(v_sb, pv[:])

            def emit_out(xr, ts0, out_sb):
                po = pout_pool.tile([P, D_MODEL], F32, tag="pout", name="po")
                for j, (r0, rw) in enumerate(R_CHUNKS):
                    for h in range(2):
                        nc.tensor.matmul(
                            po[:, h * 512 : h * 512 + 512],
                            xr[0:rw, j, ts0 : ts0 + P],
                            wr_all[0:rw, j, h * 512 : h * 512 + 512],
                            start=(j == 0),
                            stop=(j == NJ - 1),
                        )
                nc.vector.tensor_copy(out_sb, po[:])

            def pair_store(eng, dram, tl, row0):
                dst = dram[row0 : row0 + 2 * P, :].rearrange("(g p) d -> p g d", p=P)
                eng.dma_start(dst, tl[:])

            def issue_loads(it):
                sz = SLABS[it]
                t0 = sum(SLABS[:it])
                xt = inpool.tile([P, NJ, sz], mm_dt, tag="x", name="xt",
                                 padded_shape=[P, NJ, max_sz])
                wc = wcpool.tile([P, NJ, sz], mm_dt, tag="wc", name="wc",
                                 padded_shape=[P, NJ, max_sz])
                nc.sync.dma_start(
                    xt[:, 0:8, :],
                    st_d[0:1024, t0 : t0 + sz].rearrange("(j p) t -> p j t", p=P))
                nc.sync.dma_start(xt[0:64, 8, :], st_d[1024:1088, t0 : t0 + sz])
                nc.scalar.dma_start(
                    wc[:, 0:8, :],
                    wct_d[0:1024, t0 : t0 + sz].rearrange("(j p) t -> p j t", p=P))
                nc.scalar.dma_start(wc[0:64, 8, :], wct_d[1024:1088, t0 : t0 + sz])
                return xt, wc

            # preload slabs 0 and 1
            slab_tiles = {0: issue_loads(0), 1: issue_loads(1)}
            deferred_kv = []  # (row0, k_pair, v_pair) stored during slabs 2-3

            for it, sz in enumerate(SLABS):
                t0 = sum(SLABS[:it])
                ng = sz // P
                xt, wc = slab_tiles.pop(it)
                xr = xt[:]

                # prefetch next slab BEFORE this slab's stores hit the queues
                if it + 1 < len(SLABS) and it + 1 not in slab_tiles:
                    slab_tiles[it + 1] = issue_loads(it + 1)

                # flush half of the deferred ramp k/v stores per later slab
                if it >= KV_FIRST and deferred_kv:
                    nflush = (len(deferred_kv) + 1) // 2 if it < len(SLABS) - 1 \
                        else len(deferred_kv)
                    for row0, kp, vp in deferred_kv[:nflush]:
                        pair_store(nc.scalar, k_d, kp, row0)
                        pair_store(nc.sync, v_d, vp, row0)
                    deferred_kv = deferred_kv[nflush:]

                for g in range(ng):
                    sl = slice(g * P, (g + 1) * P)
                    nc.vector.tensor_add(xr[:, :, sl], xt[:, :, sl], wc[:, :, sl])

                if it < KV_FIRST:
                    # k/v first (covers the load ramp), stores deferred
                    kp = vp = None
                    for g in range(ng):
                        if g % 2 == 0:
                            kp = kv0pool.tile([P, 2, D_MODEL], out_dt, tag="k0",
                                              name="k0_pr")
                            vp = kv0pool.tile([P, 2, D_MODEL], out_dt, tag="v0",
                                              name="v0_pr")
                        emit_kv(t0 + g * P, kp[:, g % 2, :], vp[:, g % 2, :])
                        if g % 2 == 1:
                            deferred_kv.append((t0 + (g - 1) * P, kp, vp))
                    opair = None
                    for g in range(ng):
                        if g % 2 == 0:
                            opair = outpool.tile([P, 2, D_MODEL], out_dt,
                                                 tag="out", name="out_pr")
                        emit_out(xr, g * P, opair[:, g % 2, :])
                        if g % 2 == 1:
                            pair_store(nc.scalar if (g // 2) % 2 == 0 else nc.sync,
                                       out_d, opair, t0 + (g - 1) * P)
                else:
                    tiles = None
                    for g in range(ng):
                        if g % 2 == 0:
                            tiles = (
                                outpool.tile([P, 2, D_MODEL], out_dt, tag="out",
                                             name="out_pr"),
                                outpool.tile([P, 2, D_MODEL], out_dt, tag="k",
                                             name="k_pr"),
                                outpool.tile([P, 2, D_MODEL], out_dt, tag="v",
                                             name="v_pr"),
                            )
                        emit_out(xr, g * P, tiles[0][:, g % 2, :])
                        emit_kv(t0 + g * P, tiles[1][:, g % 2, :],
                                tiles[2][:, g % 2, :])
                        if g % 2 == 1:
                            row0 = t0 + (g - 1) * P
                            eng = [nc.scalar, nc.sync] if (g // 2) % 2 == 0 else \
                                  [nc.sync, nc.scalar]
                            pair_store(eng[0], out_d, tiles[0], row0)
                            pair_store(eng[1], k_d, tiles[1], row0)
                            pair_store(eng[0], v_d, tiles[2], row0)

    nc.compile()
    return nc


def _get_nc(**kw):
    key = tuple(sorted(kw.items()))
    if key not in _NC_CACHE:
        _NC_CACHE[key] = build_nc(**kw)
    return _NC_CACHE[key]


def make_in_maps(S, w_spoke, w_hub_priv, w_hub_shared, tag, W_read, cache, latent,
                 Wk, Wv):
    # host-side layout prep only (shard over batch, feature-major transposes,
    # bf16 downcast)
    bf = ml_dtypes.bfloat16
    wcat = np.concatenate(
        [np.asarray(w_spoke, np.float32), np.asarray(w_hub_priv, np.float32),
         np.asarray(w_hub_shared, np.float32), np.asarray(tag, np.float32)],
        axis=-1,
    )
    sT = np.ascontiguousarray(np.asarray(S, np.float32).transpose(0, 2, 1)).astype(bf)
    wcT = np.ascontiguousarray(wcat.transpose(0, 2, 1)).astype(bf)
    latT = np.ascontiguousarray(
        np.asarray(latent, np.float32).transpose(0, 2, 1)).astype(bf)
    wrt = np.ascontiguousarray(np.asarray(W_read, np.float32).T).astype(bf)
    wkvt = np.ascontiguousarray(
        np.stack([np.asarray(Wk, np.float32).T, np.asarray(Wv, np.float32).T],
                 axis=1)).astype(bf)
    return [
        {"st": sT[i], "wct": wcT[i], "latt": latT[i], "wrt": wrt, "wkvt": wkvt}
        for i in range(N_CORES)
    ]


def kernel(S, w_spoke, w_hub_priv, w_hub_shared, tag, W_read, cache, latent, Wk, Wv,
           **build_kw):
    in_maps = make_in_maps(S, w_spoke, w_hub_priv, w_hub_shared, tag, W_read, cache,
                           latent, Wk, Wv)
    nc = _get_nc(**build_kw)
    res = run_bass_kernel_spmd(nc, in_maps, list(range(N_CORES)))
    out = np.stack([res.results[i]["out"].astype(np.float32) for i in range(N_CORES)])
    k = np.stack([res.results[i]["k"].astype(np.float32) for i in range(N_CORES)])
    v = np.stack([res.results[i]["v"].astype(np.float32) for i in range(N_CORES)])
    return (out, k, v)
